# revision 68
# baseline (speedup 1.0000x reference)
"""Trainium2 Bass kernel for batched per-feature cubic B-spline evaluation.

Math: per feature i, sigma = 24*x in [0,24); two-sided truncated-power rep
centered at 12:  y = p(sigma) + sum_j w_j (+-(sigma-j))_+^3, j = 1..23.
All constants are pre-scaled host-side so x feeds the device ops directly
(w' = w*24^3, knot positions j/24, p in powers of x).

Custom DVE ops (registered at import into concourse.dve_ops; the cost of a
fused DVE op is the same as a simple one, so each op packs maximal math):
  KNOT_PAIR:  d = x - clamp(x, jb', jf'); out = d^2*(d*select(d>=0, wf,-wb))
              -- ONE DVE instr evaluates a fwd knot jf AND a bwd knot jb
              (their supports are disjoint); the output plane is summed
              into PSUM via an identity matmul. 9 of 11.5 pairs fit this.
  KNOT_F/B:   out = relu(+-(x-j'))^2*(w*(x-j') + c) + acc  (chained single,
              in-place accumulate for knots 12, 13)
  HORNER2:    out = (h1*x + pi1')*x + pi0'  (cubic tail; chain seed)
Pool knots (14, 10, 11): ScalarE Square |w|(s-j)^2 + ScalarE Relu
  (+-(s-j))_+ -> gpsimd tensor-tensor multiply -> per-knot diag(sign(w))
  fp32 matmul (gpsimd stt/tensor_scalar are rejected by the BIR backend).
Pairs (23,1)...(17,7) have small tails: written as fp32r planes and
accumulated with fp32r identity matmuls (1 cycle/row vs 4 for fp32;
fp32r rounds to ~11-bit mantissa -> measured norm_rel 1.30e-2 < 2e-2).
Final y = (psum + acc) on DVE in [P,1024] chunks; output DMA on the
SP/ScalarE HWDGE queues. All per-feature tables ride ONE packed DMA per
tile (separate small fp32r-typed DMA for the fp32r identity: the BIR
verifier requires an fp32r-typed producer).
Cores: 2-way feature-split x 4-way batch-split; [128, 2048] elementwise.
TimelineSim: 70248 ns (prev session's kernel: 119559; first TP kernel:
223848). Engine busy: DVE 57.8us (saturated critical path), PE 48.6,
ScalarE ~28, gpsimd ~30.
"""

import numpy as np

import concourse.bacc as bacc
import concourse.mybir as mybir
from concourse.bass_utils import run_bass_kernel_spmd
from concourse.mybir import ActivationFunctionType as AFT, AluOpType as Op
from concourse.tile import TileContext

BATCH = 8192
IN_DIM = 512
GRID_NUM = 48
K_ORD = 3
N_CORES = 8
FSHARD = 2
BSH = BATCH * FSHARD // N_CORES          # 2048 batch cols per core
FDIM = IN_DIM // FSHARD                  # 256 features per core
P = 128
NFT = FDIM // P                          # 2 feature tiles per core
NMM = 512                                # psum bank cols
NCH = BSH // NMM                         # 4 psum chunks per tile

# --- knot assignment (tunable) ----------------------------------------------
# pairs: (jf, jb) evaluated by one KNOT_PAIR DVE op -> one plane
R_PAIRS = [(23, 1), (22, 2), (21, 3), (20, 4), (19, 5), (18, 6),
           (17, 7)]                     # fp32r planes
M_PAIRS = [(16, 8), (15, 9)]            # fp32 planes
CHAIN = [('f', 12), ('f', 13)]          # chained DVE singles
POOL = [('f', 14), ('b', 10), ('b', 11)]  # ScalarE+gpsimd knots
POOL_MERGE = [(1, 2)]                   # indices into POOL merged pre-matmul
EVAC_DVE = 2                            # psum chunks evacuated on DVE (rest
                                        # via ScalarE copy + Pool add)
IO_BUFS = 2
PLANE_BUFS = 2
ACC_BUFS = 2

_CACHED_NC = None
LAST_RESULTS = None

# --- custom DVE op registration ---------------------------------------------
_OPS_REGISTERED = {}


def _register_ops():
    global _OPS_REGISTERED
    if _OPS_REGISTERED:
        return _OPS_REGISTERED
    import concourse.dve_ops as dops
    from concourse.dve_ops import DveOp, OPS, CUSTOM_DVE_SPECS, _SUB_OPCODE_FOR_NAME
    from concourse.dve_spec import (
        Spec, Src0, Src1, C0, C1, C2, C3, Zero, relu, sq, lower, maxx, minn,
        select, _spill_c3_to_src1,
    )
    from concourse.dve_uop import DveOpSpec

    def _dve_relu(x):
        return np.maximum(np.nan_to_num(x, nan=0.0, posinf=np.inf,
                                        neginf=-np.inf), 0)

    defs = []

    # HORNER2: out = (in0*in1 + c0)*in1 + c1
    defs.append(("BSP_HORNER2",
                 Spec(body=(Src0 * Src1 + C0) * Src1 + C1,
                      reference=lambda in0, in1, s0, s1, imm2:
                      ((in0.astype(np.float32) * in1 + s0) * in1 + s1)
                      .astype(np.float32))))

    # KNOT_F: u = in0 - imm2; out = relu(u)^2*(c0*u + c1) + in1
    u = Src0 - C2
    defs.append(("BSP_KNOT_F",
                 Spec(body=sq(relu(u)) * (C0 * u + C1) + Src1,
                      reference=lambda in0, in1, s0, s1, imm2:
                      (_dve_relu(in0.astype(np.float32) - imm2) ** 2
                       * (s0 * (in0 - imm2) + s1) + in1).astype(np.float32))))

    # KNOT_B: u = imm2 - in0
    ub = C2 - Src0
    defs.append(("BSP_KNOT_B",
                 Spec(body=sq(relu(ub)) * (C0 * ub + C1) + Src1,
                      reference=lambda in0, in1, s0, s1, imm2:
                      (_dve_relu(imm2 - in0.astype(np.float32)) ** 2
                       * (s0 * (imm2 - in0) + s1) + in1).astype(np.float32))))

    # KNOT_PAIR: d = in0 - clamp(in0, c3=jb, imm2=jf);
    # out = d^2 * (d * select(d>=0, c0, c1));  c0=wf, c1=-wb; in1=[P,1] jb
    m = maxx(Src0, C3)
    c = minn(m, C2)
    d = Src0 - c
    g = d >= Zero
    wsel = select(g, C0, C1)

    def _pair_ref(in0, in1, s0, s1, imm2):
        jb = in1.reshape(in0.shape[0], -1)[:, :1]
        dd = (in0.astype(np.float32)
              - np.clip(in0, jb, imm2)).astype(np.float32)
        ws = np.where(dd >= 0, s0, s1).astype(np.float32)
        return ((dd * dd) * (dd * ws)).astype(np.float32)

    defs.append(("BSP_KNOT_PAIR",
                 Spec(body=_spill_c3_to_src1((d * d) * (d * wsel)),
                      reference=_pair_ref)))

    existing = {op.name for op in OPS}
    ver = "v3"
    for name, spec in defs:
        if name in existing:
            _OPS_REGISTERED[name] = next(o for o in OPS if o.name == name)
            continue
        row = 1 + len(OPS)
        uops = lower(spec, ver=ver)
        rd1 = any(getattr(l, "sel", None) is not None and repr(l) == "Src1"
                  for l in ())
        from concourse.dve_spec import _has_src1
        tmp = DveOpSpec(name=name, opcode=row, uops=uops,
                        rd1_en=_has_src1(spec))
        sha = {ver: tmp.sha(ver), "v4": None}
        try:
            uops4 = lower(spec, ver="v4")
            tmp4 = DveOpSpec(name=name, opcode=row, uops=uops4,
                             rd1_en=_has_src1(spec))
            sha["v4"] = tmp4.sha("v4")
        except Exception:
            del sha["v4"]
        op = DveOp(name, spec, subdim=False, uops_sha=sha)
        OPS.append(op)
        CUSTOM_DVE_SPECS[name] = spec
        _SUB_OPCODE_FOR_NAME[name] = row
        _OPS_REGISTERED[name] = op
    return _OPS_REGISTERED


def _build_nc():
    ops = _register_ops()
    HORNER2 = ops["BSP_HORNER2"]
    KNOT_F = ops["BSP_KNOT_F"]
    KNOT_B = ops["BSP_KNOT_B"]
    KNOT_PAIR = ops["BSP_KNOT_PAIR"]

    cols = _prep_cols()
    NPREP = cols["_n"]

    NTBL = NPREP + P + P + len(POOL) * P
    nc = bacc.Bacc("TRN2")
    xt = nc.dram_tensor("xt", [FDIM, BSH], mybir.dt.float32,
                        kind="ExternalInput")
    tbl = nc.dram_tensor("tbl", [FDIM, NTBL], mybir.dt.float32,
                         kind="ExternalInput")
    yt = nc.dram_tensor("yt", [FDIM, BSH], mybir.dt.float32,
                        kind="ExternalOutput")

    with TileContext(nc) as tc:
        with tc.tile_pool(name="io", bufs=IO_BUFS) as io, \
             tc.tile_pool(name="pl", bufs=PLANE_BUFS) as pl, \
             tc.tile_pool(name="ac", bufs=ACC_BUFS) as ac, \
             tc.tile_pool(name="ev", bufs=4) as ev, \
             tc.tile_pool(name="ps", bufs=2, space="PSUM") as ps, \
             tc.tile_pool(name="cf", bufs=2) as cf:

            # per-tile state dicts
            T = [dict() for _ in range(NFT)]
            for ft in range(NFT):
                t = T[ft]
                fs = slice(ft * P, (ft + 1) * P)
                t["fs"] = fs
                xtile = io.tile([P, BSH], mybir.dt.float32, tag="x",
                                name=f"x{ft}")
                dmae = nc.sync if ft == 0 else nc.gpsimd
                ttile = cf.tile([P, NTBL], mybir.dt.float32, tag="tb",
                                name=f"tb{ft}")
                rtile = cf.tile([P, P], mybir.dt.float32r, tag="ir",
                                name=f"ir{ft}")
                dmae.dma_start(
                    rtile[:],
                    tbl[fs, NPREP:NPREP + P].bitcast(mybir.dt.float32r))
                t["ir"] = rtile[:]
                dmae.dma_start(ttile[:], tbl[fs, :])
                for c in range(NCH):
                    cx = slice(c * NMM, (c + 1) * NMM)
                    dmae.dma_start(xtile[:, cx], xt[fs, cx])
                ptile = ttile[:, :NPREP]
                t["x"], t["p"] = xtile, ptile
                t["i32"] = ttile[:, NPREP + P:NPREP + 2 * P]
                t["sgd"] = ttile[:, NPREP + 2 * P:]

                def col(nm, _p=ttile):
                    ci = cols[nm]
                    return _p[:, ci:ci + 1]
                t["col"] = col


            # x feeds the DVE ops directly (24-scaling folded into prep)
            half = BSH // 2
            for ft in range(NFT):
                t = T[ft]
                t["s"] = t["x"]

            # plane producers + psum accumulation, interleaved across tiles
            for ft in range(NFT):
                t = T[ft]
                t["psum"] = ps.tile([P, BSH], mybir.dt.float32,
                                    tag="ps", name=f"psum{ft}")
                t["started"] = [False] * NCH
                t["pool_cubes"] = []

            def mm_plane(t, plane, f32r, stop=False, wt=None):
                wtile = wt if wt is not None else (
                    t["ir"] if f32r else t["i32"])
                for c in range(NCH):
                    cs = slice(c * NMM, (c + 1) * NMM)
                    nc.tensor.matmul(t["psum"][:, cs], wtile,
                                     plane[:, cs],
                                     start=(not t["started"][c]),
                                     stop=stop, skip_group_check=True)
                    t["started"][c] = True

            def emit_pool_acts(t, ft, k):
                side, j = POOL[k]
                q = pl.tile([P, BSH], mybir.dt.float32, tag="q",
                            name=f"q{ft}_{j}", bufs=2)
                nc.scalar.activation(q[:], t["s"][:], AFT.Square,
                                     bias=t["col"](f"sqb{j}"),
                                     scale=t["col"](f"sqs{j}"))
                r = pl.tile([P, BSH], mybir.dt.float32, tag="r",
                            name=f"r{ft}_{j}", bufs=2)
                sc = 24.0 if side == 'f' else -24.0
                nc.scalar.activation(r[:], t["s"][:], AFT.Relu,
                                     bias=t["col"](f"rb{j}"), scale=sc)
                t.setdefault("pool_qr", []).append((q, r))

            def emit_pool_knot(t, ft, k, stop=False):
                q, r = t["pool_qr"][k]
                side, j = POOL[k]
                cube = pl.tile([P, BSH], mybir.dt.float32, tag="ct",
                               name=f"c{ft}_{j}", bufs=2)
                nc.gpsimd.tensor_tensor(cube[:], q[:], r[:], Op.mult)
                wt = t["sgd"][:, k * P:(k + 1) * P]
                mm_plane(t, cube, f32r=False, wt=wt, stop=stop)

            def emit_pair(t, ft, jf, jb, f32r, prewrite=False, stop=False,
                          halves=False):
                if prewrite:
                    for c in range(NCH):
                        hs = slice(c * NMM, (c + 1) * NMM)
                        nc.vector._custom_dve(
                            KNOT_PAIR, out=t["psum"][:, hs],
                            in0=t["s"][:, hs],
                            in1=t["col"](f"jc{jb}"), s0=t["col"](f"w{jf}"),
                            s1=t["col"](f"nw{jb}"), imm2=float(jf) / 24.0)
                    return
                dt_ = mybir.dt.float32r if f32r else mybir.dt.float32
                tag = "pr" if f32r else "pm"
                plane = pl.tile([P, BSH], dt_, tag=tag,
                                name=f"{tag}{ft}_{jf}",
                bufs=(7 if f32r else 3))
                if halves:
                    for c in range(NCH):
                        hs = slice(c * NMM, (c + 1) * NMM)
                        nc.vector._custom_dve(
                            KNOT_PAIR, out=plane[:, hs], in0=t["s"][:, hs],
                            in1=t["col"](f"jc{jb}"), s0=t["col"](f"w{jf}"),
                            s1=t["col"](f"nw{jb}"), imm2=float(jf) / 24.0)
                else:
                    nc.vector._custom_dve(
                        KNOT_PAIR, out=plane[:], in0=t["s"][:],
                        in1=t["col"](f"jc{jb}"), s0=t["col"](f"w{jf}"),
                        s1=t["col"](f"nw{jb}"), imm2=float(jf) / 24.0)
                mm_plane(t, plane, f32r=f32r, stop=stop)

            def emit_chain(t, ft, side, j):
                opk = KNOT_F if side == 'f' else KNOT_B
                nc.vector._custom_dve(
                    opk, out=t["acc"][:], in0=t["s"][:], in1=t["acc"][:],
                    s0=t["col"](f"w{j}"), s1=0.0, imm2=float(j) / 24.0)

            for k in range(len(POOL)):
                for ft in range(NFT):
                    emit_pool_acts(T[ft], ft, k)

            # schedule: psum prewrite first, all fp32r pairs next (PE
            # drains them fast, freeing DVE plane bufs), then fp32 pairs,
            # pool knots, and finally horner + chain on DVE.
            sched = [("R", 0, False), ("M", 0, False), ("R", 1, False),
                     ("P", 0, False), ("R", 2, False), ("M", 1, False),
                     ("H", 0, False), ("R", 3, False), ("C", 0, False),
                     ("P", 1, False), ("R", 4, False), ("C", 1, False),
                     ("P", 2, False), ("R", 5, False), ("R", 6, False)]
            for kind, idx, pre in sched:
                for ft in range(NFT):
                    t = T[ft]
                    if kind == "M":
                        jf, jb = M_PAIRS[idx]
                        emit_pair(t, ft, jf, jb, f32r=False, prewrite=pre)
                    elif kind == "R":
                        jf, jb = R_PAIRS[idx]
                        emit_pair(t, ft, jf, jb, f32r=True,
                                  stop=(idx == len(R_PAIRS) - 1),
                                  halves=(idx == 0))
                    elif kind == "P":
                        emit_pool_knot(t, ft, idx)
                    elif kind == "H":
                        h1 = ac.tile([P, BSH], mybir.dt.float32, tag="h1",
                                     name=f"h1_{ft}", bufs=2)
                        nc.scalar.activation(h1[:], t["s"][:], AFT.Identity,
                                             bias=t["col"]("pi2"),
                                             scale=t["col"]("pi3"))
                        acc = ac.tile([P, BSH], mybir.dt.float32, tag="acc",
                                      name=f"acc_p_{ft}")
                        nc.vector._custom_dve(
                            HORNER2, out=acc[:], in0=h1[:], in1=t["s"][:],
                            s0=t["col"]("pi1"), s1=t["col"]("pi0"),
                            imm2=0.0)
                        t["acc"] = acc
                    elif kind == "C":
                        side, j = CHAIN[idx]
                        emit_chain(t, ft, side, j)
                    elif kind == "A":
                        for c in range(NCH):
                            cs = slice(c * NMM, (c + 1) * NMM)
                            nc.tensor.matmul(t["psum"][:, cs], t["i32"][:],
                                             t["acc"][:, cs], start=False,
                                             stop=False,
                                             skip_group_check=True)

            # evac: y = psum + acc on DVE in [P,1024] chunks; DMA on
            # SP (tile0) / gpsimd (tile1) queues in parallel
            for ft in range(NFT):
                t = T[ft]
                fs = t["fs"]
                dmae = nc.sync if ft == 0 else nc.scalar
                for c in range(2):
                    cs = slice(c * half, (c + 1) * half)
                    yout = ev.tile([P, half], mybir.dt.float32, tag="yo",
                                   name=f"yo{ft}_{c}", bufs=4)
                    nc.vector.tensor_tensor(yout[:], t["psum"][:, cs],
                                            t["acc"][:, cs], Op.add)
                    for q in range(2):
                        qs = slice(q * NMM, (q + 1) * NMM)
                        ys = slice(c * half + q * NMM,
                                   c * half + (q + 1) * NMM)
                        dmae.dma_start(yt[fs, ys], yout[:, qs])
    nc.compile()
    return nc


def _prep_cols():
    """Column layout of the prep tensor."""
    cols = {}
    n = 0
    for nm in ("pi3", "pi2", "pi1", "pi0"):
        cols[nm] = n
        n += 1
    for j in range(1, 24):
        cols[f"w{j}"] = n
        n += 1
    for j in range(1, 24):
        cols[f"nw{j}"] = n
        n += 1
    for j in range(1, 12):
        cols[f"jc{j}"] = n          # constant j (bwd partner) per partition
        n += 1
    for (side, j) in POOL:
        cols[f"sqs{j}"] = n         # sqrt|w|
        cols[f"sqb{j}"] = n + 1     # -j*sqrt|w|
        cols[f"sg{j}"] = n + 2      # sign(w)
        cols[f"rb{j}"] = n + 3      # relu bias: -j (fwd) / +j (bwd)
        n += 4
    cols["_n"] = n
    return cols


def _prep_tables(coef):
    """Host-side table prep (f64)."""
    c = coef.astype(np.float64)
    NKI, KOFF = 24, 24
    C0 = c[:, KOFF:KOFF + NKI]
    C1 = c[:, KOFF + 1:KOFF + 1 + NKI]
    C2 = c[:, KOFF + 2:KOFF + 2 + NKI]
    C3 = c[:, KOFF + 3:KOFF + 3 + NKI]
    a0 = (C0 + 4 * C1 + C2) / 6
    a1 = (C2 - C0) / 2
    a2 = (C0 - 2 * C1 + C2) / 2
    a3 = (-C0 + 3 * C1 - 3 * C2 + C3) / 6

    beta0 = a0[:, 11] + a1[:, 11] + a2[:, 11] + a3[:, 11]
    beta1 = a1[:, 11] + 2 * a2[:, 11] + 3 * a3[:, 11]
    beta2 = a2[:, 11] + 3 * a3[:, 11]
    beta3 = a3[:, 11]
    w = a3[:, 1:24] - a3[:, 0:23]

    t0 = -12.0
    pi0 = beta0 + beta1 * t0 + beta2 * t0 ** 2 + beta3 * t0 ** 3
    pi1 = beta1 + 2 * beta2 * t0 + 3 * beta3 * t0 ** 2
    pi2 = beta2 + 3 * beta3 * t0
    pi3 = beta3

    cols = _prep_cols()
    prep = np.zeros((IN_DIM, cols["_n"]), np.float64)
    # x feeds ops directly: fold sigma = 24*x into the constants
    S = 24.0
    prep[:, cols["pi3"]] = pi3 * S ** 3
    prep[:, cols["pi2"]] = pi2 * S ** 2
    prep[:, cols["pi1"]] = pi1 * S
    prep[:, cols["pi0"]] = pi0
    for j in range(1, 24):
        prep[:, cols[f"w{j}"]] = w[:, j - 1] * S ** 3
        prep[:, cols[f"nw{j}"]] = -w[:, j - 1] * S ** 3
    for j in range(1, 12):
        prep[:, cols[f"jc{j}"]] = float(j) / S
    for (side, j) in POOL:
        wj = w[:, j - 1]
        prep[:, cols[f"sqs{j}"]] = S * np.sqrt(np.abs(wj))
        prep[:, cols[f"sqb{j}"]] = -float(j) * np.sqrt(np.abs(wj))
        prep[:, cols[f"sg{j}"]] = np.where(wj >= 0, 1.0, -1.0)
        prep[:, cols[f"rb{j}"]] = -float(j) if side == 'f' else float(j)
    return prep.astype(np.float32)


def kernel(x, grid, coef):
    global _CACHED_NC, LAST_RESULTS
    x = np.ascontiguousarray(np.asarray(x, dtype=np.float32))
    coef = np.asarray(coef, dtype=np.float32)
    assert x.shape == (BATCH, IN_DIM)
    assert coef.shape == (IN_DIM, GRID_NUM + K_ORD)

    prep = _prep_tables(coef)

    if _CACHED_NC is None:
        _CACHED_NC = _build_nc()
    nc = _CACHED_NC

    xT = np.ascontiguousarray(x.T)
    nbs = N_CORES // FSHARD
    ident = np.zeros((FDIM, P), np.float32)
    ident[np.arange(FDIM), np.arange(FDIM) % P] = 1.0
    c64 = coef.astype(np.float64)
    C0_, C1_, C2_, C3_ = (c64[:, 24:48], c64[:, 25:49], c64[:, 26:50],
                          c64[:, 27:51])
    a3_ = (-C0_ + 3 * C1_ - 3 * C2_ + C3_) / 6
    w_ = a3_[:, 1:24] - a3_[:, 0:23]
    sgn_all = np.zeros((IN_DIM, len(POOL) * P), np.float32)
    rows = np.arange(IN_DIM)
    colp = rows % P
    for k, (side, j) in enumerate(POOL):
        sgn_all[rows, k * P + colp] = np.where(w_[:, j - 1] >= 0, 1.0, -1.0)
    in_maps = []
    for cidx in range(N_CORES):
        fi, bj = cidx // nbs, cidx % nbs
        fsl = slice(fi * FDIM, (fi + 1) * FDIM)
        tblv = np.concatenate(
            [prep[fsl], ident, ident, sgn_all[fsl]], axis=1)
        im = {"xt": np.ascontiguousarray(
                  xT[fsl, bj * BSH:(bj + 1) * BSH]),
              "tbl": np.ascontiguousarray(tblv)}
        in_maps.append(im)
    res = run_bass_kernel_spmd(nc, in_maps, core_ids=list(range(N_CORES)))
    LAST_RESULTS = res

    y = np.empty((BATCH, IN_DIM), np.float32)
    for cidx in range(N_CORES):
        fi, bj = cidx // nbs, cidx % nbs
        y[bj * BSH:(bj + 1) * BSH, fi * FDIM:(fi + 1) * FDIM] = \
            res.results[cidx]["yt"].T
    return y
